# revision 1
# baseline (speedup 1.0000x reference)
"""Balanced grouped-expert SwiGLU kernel, v3.

Tokens are tile-balanced across 8 cores; each core's slots span at most two
experts (A then B, A = the larger run). Slots 0..1 are compiled
unconditionally against weight set A and the last slot against set B, so
only middle slots pay the If/Else scheduling-unit cost. x loads and output
stores are hoisted out of the If bodies; the B weight set streams in after
slot 1 issues; slot 0's stage 1 runs k-outer so the PE starts as soon as the
first weight/x chunks land. Output is stored bf16 and upcast on host.
"""

import math
import os

import ml_dtypes
import numpy as np

D = 2048
F = 512
MT = 256
TS = MT // 128
KC = D // 128
FC = F // 128
NCORES = 8

_cache = {}


def _build(nt: int, naf: int | None = None):
    import concourse.bacc as bacc
    import concourse.mybir as mybir
    from concourse.tile import TileContext

    dt = mybir.dt
    f32 = dt.float32
    bf16 = dt.bfloat16
    i32 = dt.int32
    PAD_T = nt * MT

    nc = bacc.Bacc(
        "TRN2", target_bir_lowering=False, debug=False,
        enable_asserts=False, num_devices=NCORES,
    )

    xpt = nc.dram_tensor("xpt", [D, PAD_T], bf16, kind="ExternalInput")
    wa1 = nc.dram_tensor("wa1", [D, F], bf16, kind="ExternalInput")
    wa2 = nc.dram_tensor("wa2", [F, D], bf16, kind="ExternalInput")
    wa3 = nc.dram_tensor("wa3", [D, F], bf16, kind="ExternalInput")
    wb1 = nc.dram_tensor("wb1", [D, F], bf16, kind="ExternalInput")
    wb2 = nc.dram_tensor("wb2", [F, D], bf16, kind="ExternalInput")
    wb3 = nc.dram_tensor("wb3", [D, F], bf16, kind="ExternalInput")
    meta = nc.dram_tensor("meta", [1, 1], i32, kind="ExternalInput")
    out = nc.dram_tensor("out", [PAD_T, D], bf16, kind="ExternalOutput")

    # After the host swaps the larger run into A, nB <= floor(nt/2), so
    # real-B tiles only ever occupy slots >= ceil(nt/2): slots below that are
    # A or idle and can skip the If entirely. The host may pass a larger
    # bound (nt - max_nB) computed from the actual counts.
    n_a_fixed = max(1, math.ceil(nt / 2)) if nt >= 2 else nt
    if naf is not None:
        n_a_fixed = max(n_a_fixed, min(naf, nt - 1))
    has_b_slot = nt >= 2

    with TileContext(nc) as tc:
        with (
            tc.tile_pool(name="wpool", bufs=1) as wpool,
            tc.tile_pool(name="xt", bufs=12) as xt_pool,
            tc.tile_pool(name="ht", bufs=2) as ht_pool,
            tc.tile_pool(name="sil", bufs=3) as sil_pool,
            tc.tile_pool(name="osb", bufs=2) as osb_pool,
            tc.tile_pool(name="ps", bufs=8, space="PSUM") as ps_pool,
        ):
            def load_x_group(m, g):
                xt = xt_pool.tile([128, 4, MT], bf16, tag="xt")
                src = xpt.ap()[g * 512:(g + 1) * 512, m * MT:(m + 1) * MT]
                nc.sync.dma_start(
                    out=xt[:],
                    in_=src.rearrange("(c p) t -> p c t", p=128),
                )
                return xt

            def load_x(m):
                return [load_x_group(m, g) for g in range(4)]

            wsb = {}
            # Set A w1/w3 chunks interleaved with slot-0 x groups in k order,
            # matching the k-outer consumption order of slot 0's stage 1.
            sa1 = wpool.tile([128, KC, F], bf16, tag="w1a")
            sa3 = wpool.tile([128, KC, F], bf16, tag="w3a")
            r1 = wa1.ap().rearrange("(k p) f -> p k f", p=128)
            r3 = wa3.ap().rearrange("(k p) f -> p k f", p=128)
            pair0 = nt >= 2 and n_a_fixed > 1
            xts0 = []
            for kk in range(4):
                if kk == 0:
                    # split the first chunk at single-k granularity so the
                    # first matmul only waits for ~0.4 MB of DMA
                    nc.sync.dma_start(out=sa1[:, 0:1, :], in_=r1[:, 0:1, :])
                    nc.sync.dma_start(out=sa3[:, 0:1, :], in_=r3[:, 0:1, :])
                    w0 = 2 * MT if pair0 else MT
                    xt0 = xt_pool.tile([128, 4, w0], bf16, tag="xt")
                    nc.sync.dma_start(
                        out=xt0[:, 0:1, :],
                        in_=xpt.ap()[0:128, 0:w0].rearrange(
                            "(c p) t -> p c t", p=128),
                    )
                    nc.sync.dma_start(out=sa1[:, 1:4, :], in_=r1[:, 1:4, :])
                    nc.sync.dma_start(out=sa3[:, 1:4, :], in_=r3[:, 1:4, :])
                    nc.sync.dma_start(
                        out=xt0[:, 1:4, :],
                        in_=xpt.ap()[128:512, 0:w0].rearrange(
                            "(c p) t -> p c t", p=128),
                    )
                    xts0.append(xt0)
                    continue
                sl = slice(kk * 4, (kk + 1) * 4)
                nc.sync.dma_start(out=sa1[:, sl, :], in_=r1[:, sl, :])
                nc.sync.dma_start(out=sa3[:, sl, :], in_=r3[:, sl, :])
                if pair0:
                    xtp = xt_pool.tile([128, 4, 2 * MT], bf16, tag="xt")
                    src = xpt.ap()[kk * 512:(kk + 1) * 512, 0:2 * MT]
                    nc.sync.dma_start(out=xtp[:],
                                      in_=src.rearrange("(c p) t -> p c t", p=128))
                    xts0.append(xtp)
                else:
                    xts0.append(load_x_group(0, kk))
            sa2 = wpool.tile([128, FC, D], bf16, tag="w2a")
            nc.sync.dma_start(out=sa2[:], in_=wa2.ap().rearrange("(c p) d -> p c d", p=128))
            wsb["a"] = (sa1, sa2, sa3)
            xts_pre = {0: xts0}
            if nt > 1 and not pair0:
                xts_pre[1] = load_x(1)

            # meta: only the branched slots (>= ceil(nt/2)) need it.
            msb = wpool.tile([1, 1], i32, tag="meta")
            nc.sync.dma_start(out=msb[:], in_=meta.ap())
            ta_v = nc.snap(nc.values_load(msb[0:1, 0:1]))

            def load_b():
                sb1 = wpool.tile([128, KC, F], bf16, tag="w1b")
                sb3 = wpool.tile([128, KC, F], bf16, tag="w3b")
                sb2 = wpool.tile([128, FC, D], bf16, tag="w2b")
                nc.sync.dma_start(out=sb1[:], in_=wb1.ap().rearrange("(k p) f -> p k f", p=128))
                nc.sync.dma_start(out=sb3[:], in_=wb3.ap().rearrange("(k p) f -> p k f", p=128))
                nc.sync.dma_start(out=sb2[:], in_=wb2.ap().rearrange("(c p) d -> p c d", p=128))
                wsb["b"] = (sb1, sb2, sb3)

            def tile_body(m, which, xts, osbs, korder=False):
                w1_sb, w2_sb, w3_sb = wsb[which]
                ht = ht_pool.tile([128, FC, MT], bf16, tag="ht")
                if korder:
                    # k-outer: consumes x/w chunks in arrival order so the
                    # first slot's compute pipelines with the prologue DMAs.
                    # Uses all 8 PSUM banks.
                    x1ts, x3ts = [], []
                    for f in range(FC):
                        x1t = ps_pool.tile([128, MT], f32, tag="ps")
                        x3t = ps_pool.tile([128, MT], f32, tag="ps")
                        x1ts.append(x1t)
                        x3ts.append(x3t)
                    for k in range(KC):
                        rhs = xts[k // 4][:, k % 4, :]
                        for f in range(FC):
                            lhs1 = w1_sb[:, k, f * 128:(f + 1) * 128]
                            lhs3 = w3_sb[:, k, f * 128:(f + 1) * 128]
                            nc.tensor.matmul(x1ts[f][:], lhs1, rhs,
                                             start=(k == 0), stop=(k == KC - 1))
                            nc.tensor.matmul(x3ts[f][:], lhs3, rhs,
                                             start=(k == 0), stop=(k == KC - 1))
                    for f in range(FC):
                        x1t, x3t = x1ts[f], x3ts[f]
                        sig = sil_pool.tile([128, MT], f32, tag="sig")
                        nc.scalar.activation(sig[:], x1t[:],
                                             mybir.ActivationFunctionType.Sigmoid)
                        sil = sil_pool.tile([128, MT], f32, tag="sil")
                        nc.vector.tensor_mul(sil[:], x1t[:], sig[:])
                        nc.vector.tensor_mul(ht[:, f, :], sil[:], x3t[:])
                else:
                    for f in range(FC):
                        x1t = ps_pool.tile([128, MT], f32, tag="ps")
                        x3t = ps_pool.tile([128, MT], f32, tag="ps")
                        for k in range(KC):
                            lhs1 = w1_sb[:, k, f * 128:(f + 1) * 128]
                            lhs3 = w3_sb[:, k, f * 128:(f + 1) * 128]
                            rhs = xts[k // 4][:, k % 4, :]
                            nc.tensor.matmul(x1t[:], lhs1, rhs,
                                             start=(k == 0), stop=(k == KC - 1))
                            nc.tensor.matmul(x3t[:], lhs3, rhs,
                                             start=(k == 0), stop=(k == KC - 1))
                        sig = sil_pool.tile([128, MT], f32, tag="sig")
                        nc.scalar.activation(sig[:], x1t[:],
                                             mybir.ActivationFunctionType.Sigmoid)
                        sil = sil_pool.tile([128, MT], f32, tag="sil")
                        nc.vector.tensor_mul(sil[:], x1t[:], sig[:])
                        nc.vector.tensor_mul(ht[:, f, :], sil[:], x3t[:])

                for ts in range(TS):
                    # 4 accumulators: each stationary ht slice feeds 4
                    # consecutive matmuls across 4 alternating PSUM banks
                    pos = []
                    for _j in range(4):
                        po = ps_pool.tile([128, 512], f32, tag="ps")
                        pos.append(po)
                    for fc in range(FC):
                        lhs = ht[:, fc, ts * 128:(ts + 1) * 128]
                        for j in range(4):
                            rj = w2_sb[:, fc, j * 512:(j + 1) * 512]
                            nc.tensor.matmul(pos[j][:], lhs, rj,
                                             start=(fc == 0), stop=(fc == FC - 1))
                    for j in range(4):
                        nc.vector.tensor_copy(
                            osbs[:, ts, j * 512:(j + 1) * 512], pos[j][:])

            def paired_body(m, xts=None, korder=False):
                # two adjacent fixed-A slots: stage 1 runs once with a
                # 2*MT-wide moving operand (halves its matmul count)
                w1_sb, w2_sb, w3_sb = wsb["a"]
                if xts is None:
                    xts = []
                    for g in range(4):
                        xt = xt_pool.tile([128, 4, 2 * MT], bf16, tag="xt")
                        src = xpt.ap()[g * 512:(g + 1) * 512, m * MT:(m + 2) * MT]
                        nc.sync.dma_start(out=xt[:],
                                          in_=src.rearrange("(c p) t -> p c t", p=128))
                        xts.append(xt)
                ht = ht_pool.tile([128, FC, 2 * MT], bf16, tag="ht")
                if korder:
                    # consume x/w chunks in arrival order so the first pair
                    # pipelines with the prologue DMA stream
                    x1ts, x3ts = [], []
                    for f in range(FC):
                        x1t = ps_pool.tile([128, 2 * MT], f32, tag="ps")
                        x3t = ps_pool.tile([128, 2 * MT], f32, tag="ps")
                        x1ts.append(x1t)
                        x3ts.append(x3t)
                    for k in range(KC):
                        rhs = xts[k // 4][:, k % 4, :]
                        for f in range(FC):
                            lhs1 = w1_sb[:, k, f * 128:(f + 1) * 128]
                            lhs3 = w3_sb[:, k, f * 128:(f + 1) * 128]
                            nc.tensor.matmul(x1ts[f][:], lhs1, rhs,
                                             start=(k == 0), stop=(k == KC - 1))
                            nc.tensor.matmul(x3ts[f][:], lhs3, rhs,
                                             start=(k == 0), stop=(k == KC - 1))
                    for f in range(FC):
                        x1t, x3t = x1ts[f], x3ts[f]
                        sig = sil_pool.tile([128, 2 * MT], f32, tag="sig")
                        nc.scalar.activation(sig[:], x1t[:],
                                             mybir.ActivationFunctionType.Sigmoid)
                        sil = sil_pool.tile([128, 2 * MT], f32, tag="sil")
                        nc.vector.tensor_mul(sil[:], x1t[:], sig[:])
                        nc.vector.tensor_mul(ht[:, f, :], sil[:], x3t[:])
                else:
                    for f in range(FC):
                        x1t = ps_pool.tile([128, 2 * MT], f32, tag="ps")
                        x3t = ps_pool.tile([128, 2 * MT], f32, tag="ps")
                        for k in range(KC):
                            lhs1 = w1_sb[:, k, f * 128:(f + 1) * 128]
                            lhs3 = w3_sb[:, k, f * 128:(f + 1) * 128]
                            rhs = xts[k // 4][:, k % 4, :]
                            nc.tensor.matmul(x1t[:], lhs1, rhs,
                                             start=(k == 0), stop=(k == KC - 1))
                            nc.tensor.matmul(x3t[:], lhs3, rhs,
                                             start=(k == 0), stop=(k == KC - 1))
                        sig = sil_pool.tile([128, 2 * MT], f32, tag="sig")
                        nc.scalar.activation(sig[:], x1t[:],
                                             mybir.ActivationFunctionType.Sigmoid)
                        sil = sil_pool.tile([128, 2 * MT], f32, tag="sil")
                        nc.vector.tensor_mul(sil[:], x1t[:], sig[:])
                        nc.vector.tensor_mul(ht[:, f, :], sil[:], x3t[:])
                for sub in range(2):
                    osb = osb_pool.tile([128, TS, D], bf16, tag="osb")
                    for ts2 in range(TS):
                        ts = sub * TS + ts2
                        # 4 accumulators: each stationary ht slice feeds 4
                        # consecutive matmuls across 4 alternating PSUM banks
                        pos = []
                        for _j in range(4):
                            po = ps_pool.tile([128, 512], f32, tag="ps")
                            pos.append(po)
                        for fc in range(FC):
                            lhs = ht[:, fc, ts * 128:(ts + 1) * 128]
                            for j in range(4):
                                rj = w2_sb[:, fc, j * 512:(j + 1) * 512]
                                nc.tensor.matmul(pos[j][:], lhs, rj,
                                                 start=(fc == 0),
                                                 stop=(fc == FC - 1))
                        for j in range(4):
                            nc.vector.tensor_copy(
                                osb[:, ts2, j * 512:(j + 1) * 512], pos[j][:])
                    mm2 = m + sub
                    for ts2 in range(TS):
                        nc.sync.dma_start(
                            out=out[mm2 * MT + ts2 * 128: mm2 * MT + (ts2 + 1) * 128, :],
                            in_=osb[:, ts2, :],
                        )

            m = 0
            while m < nt:
                # pair adjacent fixed-A slots
                if m % 2 == 0 and m + 1 < n_a_fixed:
                    paired_body(m, xts=xts_pre.get(m), korder=(m == 0))
                    if m == (0 if n_a_fixed < 4 else 2) and has_b_slot:
                        load_b()
                    m += 2
                    continue
                xts = xts_pre[m] if m in xts_pre else load_x(m)
                osb = osb_pool.tile([128, TS, D], bf16, tag="osb")
                if m < n_a_fixed:
                    # guaranteed expert A by the host assignment
                    tile_body(m, "a", xts, osb, korder=(m == 0))
                    if m == min(2, n_a_fixed - 1, nt - 2) and has_b_slot \
                            and not (2 >= 2 and 2 % 2 == 0 and 3 < n_a_fixed):
                        load_b()
                elif has_b_slot and m == nt - 1:
                    # guaranteed expert B (wb==wa for single-expert cores)
                    tile_body(m, "b", xts, osb)
                else:
                    with tc.If(ta_v > m) as cmp:
                        tile_body(m, "a", xts, osb)
                    with cmp.Else():
                        tile_body(m, "b", xts, osb)
                for ts in range(TS):
                    nc.sync.dma_start(
                        out=out[m * MT + ts * 128: m * MT + (ts + 1) * 128, :],
                        in_=osb[:, ts, :],
                    )
                m += 1

    nc.compile()
    return nc


def _get_program(nt: int, naf: int | None = None):
    key = (nt, naf)
    if key not in _cache:
        _cache[key] = _build(nt, naf)
    return _cache[key]


def _assign(counts, nt_cap=None):
    """Greedy: chunk the padded-tile list into per-core runs of <=NT tiles
    spanning <=2 experts. Returns (nt, per-core list of (expert, tile_lo,
    n_tiles) segment pairs) or None if infeasible."""
    E = len(counts)
    pt = [max(1, math.ceil(c / MT)) if c > 0 else 0 for c in counts]
    total = sum(pt)
    nt = math.ceil(total / NCORES)
    for nt_try in (nt, nt + 1):
        segs = [[] for _ in range(NCORES)]
        e, used = 0, 0
        for c in range(NCORES):
            cap = nt_try
            nexp = 0
            while cap > 0 and e < E:
                if pt[e] - used == 0:
                    e += 1
                    used = 0
                    continue
                if nexp == 2:
                    break
                take = min(cap, pt[e] - used)
                segs[c].append((e, used, take))
                used += take
                cap -= take
                nexp += 1
        leftover = total - sum(s[2] for core in segs for s in core)
        if leftover == 0:
            return nt_try, segs
    return None


def kernel(x, num_tokens_per_expert, w1, w2, w3):
    from concourse.bass_utils import run_bass_kernel_spmd

    x = np.asarray(x)
    counts = [int(v) for v in np.asarray(num_tokens_per_expert)]
    w1 = np.asarray(w1)
    w2 = np.asarray(w2)
    w3 = np.asarray(w3)
    T, E = x.shape[0], len(counts)
    starts = np.concatenate([[0], np.cumsum(counts)])[:E].astype(np.int64)

    plan = _assign(counts)
    if plan is None:
        # fallback: expert-parallel (1 segment per core), padded to max tiles
        pt = [max(1, math.ceil(c / MT)) if c > 0 else 0 for c in counts]
        nt = max(pt)
        segs = [[(e, 0, pt[e])] if pt[e] else [] for e in range(min(E, NCORES))]
        segs += [[] for _ in range(NCORES - len(segs))]
        plan = (nt, segs)
    nt, segs = plan
    nt = max(nt, 2)
    # pre-swap so the larger run is A, then size the fixed-A region to the
    # largest remaining B run
    segs = [([s[1], s[0]] if len(s) == 2 and s[1][2] > s[0][2] else list(s))
            for s in segs]
    max_nb = max((s[1][2] for s in segs if len(s) == 2), default=0)
    naf = min(nt - max_nb, nt - 1)
    nc = _get_program(nt, naf)
    PAD_T = nt * MT

    w1b = w1.astype(ml_dtypes.bfloat16)
    w2b = w2.astype(ml_dtypes.bfloat16)
    w3b = w3.astype(ml_dtypes.bfloat16)
    xT = np.ascontiguousarray(x.T).astype(ml_dtypes.bfloat16)  # [D, T]

    in_maps = []
    placements = []  # per core: list of (slot, src_lo, n_rows)
    for c in range(NCORES):
        cs = list(segs[c])
        if len(cs) == 2:
            na, nb = cs[0][2], cs[1][2]
            slot_base = [0, nt - nb]
            ta = na
            ea, eb = cs[0][0], cs[1][0]
        elif len(cs) == 1:
            na, nb = cs[0][2], 0
            slot_base = [0]
            ta = nt
            ea = eb = cs[0][0]
        else:
            na = nb = 0
            slot_base = []
            ta = nt
            ea = eb = 0

        xpt = np.zeros((D, PAD_T), dtype=ml_dtypes.bfloat16)
        place = []
        for si, (e, tile_lo, ntk) in enumerate(cs):
            src_lo = int(starts[e]) + tile_lo * MT
            src_hi = min(int(starts[e]) + counts[e], src_lo + ntk * MT)
            nrow = src_hi - src_lo
            lo = slot_base[si] * MT
            xpt[:, lo: lo + nrow] = xT[:, src_lo:src_hi]
            place.append((slot_base[si], src_lo, nrow))
        placements.append(place)
        in_maps.append({
            "xpt": xpt,
            "wa1": np.ascontiguousarray(w1b[ea]),
            "wa2": np.ascontiguousarray(w2b[ea]),
            "wa3": np.ascontiguousarray(w3b[ea]),
            "wb1": np.ascontiguousarray(w1b[eb]),
            "wb2": np.ascontiguousarray(w2b[eb]),
            "wb3": np.ascontiguousarray(w3b[eb]),
            "meta": np.array([[ta]], dtype=np.int32),
        })

    trace = bool(int(os.environ.get("KERNEL_TRACE", "0")))
    try:
        res = run_bass_kernel_spmd(nc, in_maps, core_ids=list(range(NCORES)),
                                   trace=trace)
    except ModuleNotFoundError:
        res = run_bass_kernel_spmd(nc, in_maps, core_ids=list(range(NCORES)),
                                   trace=False)
    kernel.last_results = res

    out = np.empty((T, D), dtype=np.float32)
    for c in range(NCORES):
        o = np.asarray(res.results[c]["out"])
        for (slot, src_lo, nrow) in placements[c]:
            out[src_lo:src_lo + nrow] = o[slot * MT: slot * MT + nrow].astype(
                np.float32)
    return out


if __name__ == "__main__":
    import simbench
    nc = _build(9)
    simbench.run(nc)

